# revision 2
# baseline (speedup 1.0000x reference)
"""Batched GNN neighbor aggregation on 8 NeuronCores.

out[b] = neibors[b] @ last_embs[b]  for b in 0..7  (2048x2048 @ 2048x128, f32)

Sharding: one graph per core (batch dim across the 8 cores), no cross-core
communication. The PE contracts over the partition dimension, so the
adjacency operand must land in SBUF with the contraction index (m) on
partitions; we pre-transpose each graph's adjacency on the host during
sharding so the device streams it with fully-contiguous 1MB DMAs and does
nothing but matmuls. The device computes out^T = embs^T @ neibors^T with
the embedding K-chunks stationary; the host transposes the (small) result
back.
"""

import numpy as np

B = 8      # graphs == cores
N = 2048   # nodes per graph
D = 128    # embedding size
KT = 128   # contraction tile (partition dim)
NT = 512   # node tile (one f32 PSUM bank)
NK = N // KT  # 16
NN = N // NT  # 4

_cached_nc = None


def _build_program():
    import concourse.tile as tile
    from concourse import bacc, mybir

    f32 = mybir.dt.float32
    nc = bacc.Bacc("TRN2", target_bir_lowering=False, debug=False)

    # at[m, n] = neibors[g][n, m] (host-transposed adjacency)
    at = nc.dram_tensor("at", [N, N], f32, kind="ExternalInput")
    # embs[p, k, d] = last_embs[g][k*128 + p, d] (host-rearranged)
    embs = nc.dram_tensor("embs", [KT, NK, D], f32, kind="ExternalInput")
    # out_t[d, n] = out[g][n, d]
    out_t = nc.dram_tensor("out_t", [D, N], f32, kind="ExternalOutput")

    with tile.TileContext(nc) as tc:
        with (
            tc.tile_pool(name="econst", bufs=1) as epool,
            tc.tile_pool(name="at", bufs=6) as apool,
            tc.tile_pool(name="psum", bufs=1, space="PSUM") as pspool,
            tc.tile_pool(name="out", bufs=1) as opool,
        ):
            e_sb = epool.tile([KT, NK, D], f32)
            nc.sync.dma_start(e_sb[:], embs.ap()[:])

            ps = [
                pspool.tile([D, NT], f32, name=f"ps{n}", tag=f"ps{n}")
                for n in range(NN)
            ]

            for k in range(NK):
                strip = apool.tile([KT, N], f32, tag="strip")
                nc.sync.dma_start(strip[:], at.ap()[k * KT : (k + 1) * KT, :])
                for n in range(NN):
                    nc.tensor.matmul(
                        ps[n][:],
                        e_sb[:, k, :],
                        strip[:, n * NT : (n + 1) * NT],
                        start=(k == 0),
                        stop=(k == NK - 1),
                    )

            o_sb = opool.tile([D, N], f32)
            for n in range(NN):
                nc.vector.tensor_copy(o_sb[:, n * NT : (n + 1) * NT], ps[n][:])
            nc.sync.dma_start(out_t.ap()[:], o_sb[:])

    nc.compile()
    return nc


def _make_in_maps(last_embs, neibors):
    in_maps = []
    for g in range(B):
        at_g = np.ascontiguousarray(neibors[g].T).astype(np.float32, copy=False)
        e_g = np.ascontiguousarray(
            last_embs[g].reshape(NK, KT, D).transpose(1, 0, 2)
        ).astype(np.float32, copy=False)
        in_maps.append({"at": at_g, "embs": e_g})
    return in_maps


def _gather(results):
    out = np.stack([results[g]["out_t"].T for g in range(B)], axis=0)
    return np.ascontiguousarray(out).astype(np.float32, copy=False)


def kernel(last_embs, neibors):
    global _cached_nc
    from concourse.bass_utils import run_bass_kernel_spmd

    last_embs = np.asarray(last_embs, dtype=np.float32)
    neibors = np.asarray(neibors, dtype=np.float32)
    if _cached_nc is None:
        _cached_nc = _build_program()
    res = run_bass_kernel_spmd(
        _cached_nc, _make_in_maps(last_embs, neibors), list(range(B))
    ).results
    return _gather(res)


# revision 3
# speedup vs baseline: 1.1368x; 1.1368x over previous
"""Batched GNN neighbor aggregation on 8 NeuronCores.

out[b] = neibors[b] @ last_embs[b]  for b in 0..7  (2048x2048 @ 2048x128, f32)

Sharding: one graph per core (batch dim across the 8 cores), no cross-core
communication.

Device-side math: the PE contracts over the partition dimension, so the
adjacency operand must sit in SBUF with the contraction index (m) on
partitions; we pre-transpose each graph's adjacency on the host during
sharding so the device streams it with fully-contiguous 1MB DMAs.

Precision: TRN2's native fp32 matmul runs LOW/HIGH two-pass per operand
pair (~19 TF/s), which is slower than the HBM stream. Instead each f32
operand is split on the host into bf16 hi + bf16 lo (hi = RNE-rounded
bf16, lo = bf16 of the exact residual), and the device computes
  out = Ah@Eh + Ah@El + Al@Eh
in three bf16 passes accumulated in fp32 PSUM. The dropped Al@El term is
O(2^-18) relative, so the result keeps fp32-class accuracy while moving
the same 4 bytes/element over HBM and running the PE at bf16 rate.

The device computes out^T = embs^T @ neibors^T with the embedding K-chunks
stationary; the host transposes the (small) result back.
"""

import numpy as np
import ml_dtypes

BF16 = ml_dtypes.bfloat16

B = 8      # graphs == cores
N = 2048   # nodes per graph
D = 128    # embedding size
KT = 128   # contraction tile (partition dim)
NT = 512   # node tile (one f32 PSUM bank)
NK = N // KT  # 16
NN = N // NT  # 4

_cached_nc = None


def _build_program():
    import concourse.tile as tile
    from concourse import bacc, mybir

    f32 = mybir.dt.float32
    bf16 = mybir.dt.bfloat16
    nc = bacc.Bacc("TRN2", target_bir_lowering=False, debug=False)

    # a2[k, s, p, n] = bf16 half s (0=hi, 1=lo) of neibors[g].T[k*128 + p, n]
    a2 = nc.dram_tensor("a2", [NK, 2, KT, N], bf16, kind="ExternalInput")
    # e2[s, p, k, d] = bf16 half s of last_embs[g][k*128 + p, d]
    e2 = nc.dram_tensor("e2", [2, KT, NK, D], bf16, kind="ExternalInput")
    # out_t[d, n] = out[g][n, d]
    out_t = nc.dram_tensor("out_t", [D, N], f32, kind="ExternalOutput")

    # (e_half, a_half) product passes; Al@El is dropped (O(2^-18)).
    PASSES = [(0, 0), (1, 0), (0, 1)]

    with tile.TileContext(nc) as tc:
        with (
            tc.tile_pool(name="econst", bufs=1) as epool,
            tc.tile_pool(name="at", bufs=6) as apool,
            tc.tile_pool(name="psum", bufs=1, space="PSUM") as pspool,
            tc.tile_pool(name="out", bufs=1) as opool,
        ):
            e_sb = epool.tile([KT, 2, NK, D], bf16)
            nc.sync.dma_start(e_sb[:], e2.ap().rearrange("s p k d -> p s k d"))

            ps = [
                pspool.tile([D, NT], f32, name=f"ps{n}", tag=f"ps{n}")
                for n in range(NN)
            ]

            for k in range(NK):
                strip = apool.tile([KT, 2, N], bf16, tag="strip")
                nc.sync.dma_start(
                    strip[:], a2.ap()[k].rearrange("s p n -> p s n")
                )
                for pi, (se, sa) in enumerate(PASSES):
                    for n in range(NN):
                        nc.tensor.matmul(
                            ps[n][:],
                            e_sb[:, se, k, :],
                            strip[:, sa, n * NT : (n + 1) * NT],
                            start=(k == 0 and pi == 0),
                            stop=(k == NK - 1 and pi == len(PASSES) - 1),
                        )

            o_sb = opool.tile([D, N], f32)
            for n in range(NN):
                nc.vector.tensor_copy(o_sb[:, n * NT : (n + 1) * NT], ps[n][:])
            nc.sync.dma_start(out_t.ap()[:], o_sb[:])

    nc.compile()
    return nc


def _split_hi_lo(x):
    """Exact-residual bf16 split: x ~= hi + lo with |x - hi - lo| <= 2^-18|x|."""
    hi = x.astype(BF16)
    lo = (x - hi.astype(np.float32)).astype(BF16)
    return hi, lo


def _make_in_maps(last_embs, neibors):
    in_maps = []
    for g in range(B):
        at_g = np.ascontiguousarray(neibors[g].T)  # [m, n]
        ah, al = _split_hi_lo(at_g)
        a2 = np.stack(
            [ah.reshape(NK, KT, N), al.reshape(NK, KT, N)], axis=1
        )  # [NK, 2, KT, N]
        eh, el = _split_hi_lo(np.ascontiguousarray(last_embs[g]))  # [N, D]
        e2 = np.stack(
            [eh.reshape(NK, KT, D), el.reshape(NK, KT, D)], axis=0
        )  # [2, NK, KT, D]
        e2 = np.ascontiguousarray(e2.transpose(0, 2, 1, 3))  # [2, KT, NK, D]
        in_maps.append({"a2": np.ascontiguousarray(a2), "e2": e2})
    return in_maps


def _gather(results):
    out = np.stack([results[g]["out_t"].T for g in range(B)], axis=0)
    return np.ascontiguousarray(out).astype(np.float32, copy=False)


def kernel(last_embs, neibors):
    global _cached_nc
    from concourse.bass_utils import run_bass_kernel_spmd

    last_embs = np.asarray(last_embs, dtype=np.float32)
    neibors = np.asarray(neibors, dtype=np.float32)
    if _cached_nc is None:
        _cached_nc = _build_program()
    res = run_bass_kernel_spmd(
        _cached_nc, _make_in_maps(last_embs, neibors), list(range(B))
    ).results
    return _gather(res)
